# revision 8
# baseline (speedup 1.0000x reference)
"""AQT fake-quant matmul (nn_AqtDotGeneral) on 8 TRN2 NeuronCores.

Reference semantics (per jax oracle):
    lhs_q, ls = fake_quant(lhs, axis=-1)   # per-row int8 symmetric, ls=[B,S,1]
    rhs_q, rs = fake_quant(rhs, axis=0)    # per-col int8 symmetric, rs=[1,F]
    out = (lhs_q @ rhs_q) * ls * rs

Sharding: data-parallel on flattened batch*seq rows (65536 rows -> 8192/core),
rhs replicated; contraction dim unsharded so no collectives.

On-device per core:
  - per 128-row tile: DVE abs-max reduce -> scale; quantize via the
    +1.5*2^23 magic-add (exact round-half-to-even, matching jnp.round);
    values <=127 are exact in bf16 -> bf16 matmul accumulating in f32 PSUM
    is bit-exact integer arithmetic.
  - lhs tiles are PE-transposed (K onto partitions) before the matmul.
  - epilogue: single fused (acc * row_scale) * col_scale_broadcast.
  - rhs is quantized once on device (PE transpose -> quantize -> transpose
    back), col scales broadcast to a [128,F] tile via stride-0 DMA.
"""

import os
import sys

import numpy as np

if "/opt/trn_rl_repo" not in sys.path:
    sys.path.insert(0, "/opt/trn_rl_repo")

import concourse.bass as bass
import concourse.tile as tile
from concourse import bacc, mybir
from concourse.bass_utils import run_bass_kernel_spmd
from concourse.masks import make_identity

# Problem shape (hardcoded per spec)
B, S, D, F = 4, 16384, 512, 512
N_CORES = 8
ROWS = B * S                  # 65536
SHARD = ROWS // N_CORES       # 8192
P = 128                       # partitions
N_TILES = SHARD // P          # 64 row-tiles per core
KC = D // P                   # 4 contraction chunks
QMAX = 127.0
C_MAGIC = 1.5 * 2.0**23       # round-to-int magic constant
F32 = mybir.dt.float32
BF16 = mybir.dt.bfloat16
MAX_OP = mybir.AluOpType.max
MULT_OP = mybir.AluOpType.mult
COPY_FN = mybir.ActivationFunctionType.Copy

LAST_EXEC_TIME_NS = None
LAST_RESULTS = None


def _install_ntff_hook() -> bool:
    """Provide the antenv.axon_hooks shim this image lacks, so
    run_bass_kernel_spmd(trace=True) can capture an NTFF profile."""
    import types

    try:
        from antenv.axon_hooks import get_axon_ntff_profile_hook  # noqa: F401

        return True
    except ImportError:
        pass
    try:
        import antenv
        from trn_agent_boot.trn_boot import _ntff_profile_via_ctypes

        mod = types.ModuleType("antenv.axon_hooks")
        holder = {"h": None}
        mod.set_axon_ntff_profile_hook = lambda h: holder.__setitem__("h", h)
        mod.get_axon_ntff_profile_hook = lambda: holder["h"]
        sys.modules["antenv.axon_hooks"] = mod
        antenv.axon_hooks = mod
        mod.set_axon_ntff_profile_hook(
            _ntff_profile_via_ctypes("/opt/axon/libaxon_pjrt.so")
        )
        return holder["h"] is not None
    except Exception:
        return False


def _build():
    nc = bacc.Bacc(None, target_bir_lowering=False)

    lhs_ext = nc.declare_dram_parameter("lhs", [SHARD, D], F32, isOutput=False)
    rhs_ext = nc.declare_dram_parameter("rhs", [D, F], F32, isOutput=False)
    out_ext = nc.declare_dram_parameter("out", [SHARD, F], F32, isOutput=True)

    with tile.TileContext(nc) as tc:
        with (
            tc.tile_pool(name="singles", bufs=1) as singles,
            tc.tile_pool(name="smalls", bufs=8) as smalls,
            tc.tile_pool(name="xs", bufs=4) as xs_pool,
            tc.tile_pool(name="ts", bufs=3) as ts_pool,
            tc.tile_pool(name="qs", bufs=3) as qs_pool,
            tc.tile_pool(name="qts", bufs=3) as qts_pool,
            tc.tile_pool(name="os", bufs=4) as os_pool,
            tc.tile_pool(name="psum_qt", bufs=2, space="PSUM") as psum_qt,
            tc.tile_pool(name="psum_acc", bufs=2, space="PSUM") as psum_acc,
        ):
            id_f32 = singles.tile([P, P], F32)
            make_identity(nc, id_f32)
            id_bf16 = singles.tile([P, P], BF16)
            make_identity(nc, id_bf16)

            # ---------------- one-time rhs quantization ----------------
            # w_sb[p, k, f] = W[k*128+p, f]
            w_sb = singles.tile([P, KC, F], F32)
            nc.sync.dma_start(
                out=w_sb[:], in_=rhs_ext[:].rearrange("(k p) f -> p k f", p=P)
            )
            # quantized weight, natural [K, F] layout: w_q[k][p, f] = q(W)[k*128+p, f]
            w_q = [
                singles.tile([P, F], BF16, tag=f"w_q{k}", name=f"w_q{k}")
                for k in range(KC)
            ]
            # per-column scales as a broadcast row
            sw_row = singles.tile([1, F], F32)
            # RS[p, f] = col_scale[f] for all p
            rs_bcast = singles.tile([P, F], F32)

            for fc in range(KC):  # F split into 4 chunks of 128
                ps = psum_qt.tile([P, KC, P], F32, tag="qt")
                for k in range(KC):
                    # W[K-chunk k, F-chunk fc] block transposed -> [F-part, K-free]
                    nc.tensor.transpose(
                        ps[:, k, :],
                        w_sb[:, k, fc * P : (fc + 1) * P],
                        id_f32,
                    )
                wT = singles.tile([P, KC, P], F32, tag=f"wT{fc}")
                nc.scalar.copy(wT[:], ps[:])
                amax_w = smalls.tile([P, 1], F32, tag="amax_w")
                nc.vector.tensor_reduce(
                    amax_w,
                    wT[:],
                    axis=mybir.AxisListType.XY,
                    op=MAX_OP,
                    apply_absolute_value=True,
                )
                s_w = smalls.tile([P, 1], F32, tag="s_w")
                nc.gpsimd.tensor_scalar(
                    s_w, amax_w, 1.0 / QMAX, 1e-38, MULT_OP, MAX_OP
                )
                ivs_w = smalls.tile([P, 1], F32, tag="ivs_w")
                nc.vector.reciprocal(ivs_w, s_w)
                t_w = ts_pool.tile([P, KC, P], F32, tag="t1")
                nc.scalar.activation(
                    t_w[:], wT[:], COPY_FN, bias=C_MAGIC, scale=ivs_w
                )
                q_wT = qs_pool.tile([P, KC, P], BF16, tag="q")
                nc.gpsimd.tensor_scalar_sub(q_wT[:], t_w[:], C_MAGIC)
                for k in range(KC):
                    psb = psum_acc.tile([P, P], BF16, tag="acc")
                    nc.tensor.transpose(psb[:], q_wT[:, k, :], id_bf16)
                    nc.scalar.copy(w_q[k][:, fc * P : (fc + 1) * P], psb[:])
                # col-scale row chunk: [128,1] -> [1,128]
                ps_row = psum_acc.tile([1, P], F32, tag="acc")
                nc.tensor.transpose(ps_row[:], s_w, id_f32)
                nc.scalar.copy(sw_row[0:1, fc * P : (fc + 1) * P], ps_row[:])

            # broadcast col scales across partitions: ones[1,128].T @ sw_row[1,F]
            ones_row = singles.tile([1, P], F32)
            nc.vector.memset(ones_row[:], 1.0)
            ps_bc = psum_acc.tile([P, F], F32, tag="acc")
            nc.tensor.matmul(ps_bc[:], ones_row[:], sw_row[:], start=True, stop=True)
            nc.scalar.copy(rs_bcast[:], ps_bc[:])

            # ---------------- main loop: 64 row-tiles ----------------
            for i in range(N_TILES):
                x = xs_pool.tile([P, D], F32, tag="x")
                nc.sync.dma_start(out=x[:], in_=lhs_ext[i * P : (i + 1) * P, :])

                amax = smalls.tile([P, 1], F32, tag="amax")
                nc.vector.tensor_reduce(
                    amax,
                    x[:],
                    axis=mybir.AxisListType.X,
                    op=MAX_OP,
                    apply_absolute_value=True,
                )
                s = smalls.tile([P, 1], F32, tag="s")
                nc.gpsimd.tensor_scalar(s, amax, 1.0 / QMAX, 1e-38, MULT_OP, MAX_OP)
                ivs = smalls.tile([P, 1], F32, tag="ivs")
                nc.vector.reciprocal(ivs, s)

                t1 = ts_pool.tile([P, D], F32, tag="t1")
                nc.scalar.activation(t1[:], x[:], COPY_FN, bias=C_MAGIC, scale=ivs)
                q = qs_pool.tile([P, D], BF16, tag="q")
                nc.gpsimd.tensor_scalar_sub(q[:], t1[:], C_MAGIC)

                qt_ps = psum_qt.tile([P, KC, P], BF16, tag="qt")
                for k in range(KC):
                    nc.tensor.transpose(
                        qt_ps[:, k, :], q[:, k * P : (k + 1) * P], id_bf16
                    )
                qt = qts_pool.tile([P, KC, P], BF16, tag="qt_sb")
                for k in range(KC):
                    nc.scalar.copy(qt[:, k, :], qt_ps[:, k, :])

                acc = psum_acc.tile([P, F], F32, tag="acc")
                for k in range(KC):
                    nc.tensor.matmul(
                        acc[:],
                        qt[:, k, :],
                        w_q[k][:],
                        start=(k == 0),
                        stop=(k == KC - 1),
                    )

                o = os_pool.tile([P, F], F32, tag="o")
                nc.vector.scalar_tensor_tensor(
                    o[:], acc[:], s, rs_bcast[:], MULT_OP, MULT_OP
                )
                nc.sync.dma_start(out=out_ext[i * P : (i + 1) * P, :], in_=o[:])

    nc.compile()
    return nc


_NC_CACHE = None


def kernel(lhs: np.ndarray, rhs: np.ndarray) -> np.ndarray:
    global LAST_EXEC_TIME_NS, LAST_RESULTS, _NC_CACHE

    lhs = np.ascontiguousarray(np.asarray(lhs, dtype=np.float32))
    rhs = np.ascontiguousarray(np.asarray(rhs, dtype=np.float32))
    flat = lhs.reshape(ROWS, D)

    if _NC_CACHE is None:
        _NC_CACHE = _build()
    nc = _NC_CACHE

    in_maps = [
        {
            "lhs": np.ascontiguousarray(flat[i * SHARD : (i + 1) * SHARD]),
            "rhs": rhs,
        }
        for i in range(N_CORES)
    ]

    trace = bool(os.environ.get("KERNEL_TRACE"))
    if trace:
        trace = _install_ntff_hook()
    res = run_bass_kernel_spmd(
        nc, in_maps, core_ids=list(range(N_CORES)), trace=trace
    )
    LAST_EXEC_TIME_NS = res.exec_time_ns
    LAST_RESULTS = res

    out = np.concatenate([res.results[i]["out"] for i in range(N_CORES)], axis=0)
    return out.reshape(B, S, F).astype(np.float32)


# revision 10
# speedup vs baseline: 2.8157x; 2.8157x over previous
"""AQT fake-quant matmul (nn_AqtDotGeneral) on 8 TRN2 NeuronCores.

Reference semantics (per jax oracle):
    lhs_q, ls = fake_quant(lhs, axis=-1)   # per-row int8 symmetric, ls=[B,S,1]
    rhs_q, rs = fake_quant(rhs, axis=0)    # per-col int8 symmetric, rs=[1,F]
    out = (lhs_q @ rhs_q) * ls * rs

Sharding: data-parallel on flattened batch*seq rows (65536 rows -> 8192/core),
rhs replicated; contraction dim unsharded so no collectives.

On-device per core:
  - per 128-row tile: DVE abs-max reduce -> scale; quantize via the
    +1.5*2^23 magic-add (exact round-half-to-even, matching jnp.round);
    values <=127 are exact in bf16 -> bf16 matmul accumulating in f32 PSUM
    is bit-exact integer arithmetic.
  - lhs tiles are PE-transposed (K onto partitions) before the matmul.
  - epilogue: single fused (acc * row_scale) * col_scale_broadcast.
  - rhs is quantized once on device (PE transpose -> quantize -> transpose
    back), col scales broadcast to a [128,F] tile via stride-0 DMA.
"""

import os
import sys

import numpy as np

if "/opt/trn_rl_repo" not in sys.path:
    sys.path.insert(0, "/opt/trn_rl_repo")

import concourse.bass as bass
import concourse.tile as tile
from concourse import bacc, mybir
from concourse.bass_utils import run_bass_kernel_spmd
from concourse.masks import make_identity

# Problem shape (hardcoded per spec)
B, S, D, F = 4, 16384, 512, 512
N_CORES = 8
ROWS = B * S                  # 65536
SHARD = ROWS // N_CORES       # 8192
P = 128                       # partitions
N_TILES = SHARD // P          # 64 row-tiles per core
KC = D // P                   # 4 contraction chunks
QMAX = 127.0
C_MAGIC = 1.5 * 2.0**23       # round-to-int magic constant
F32 = mybir.dt.float32
BF16 = mybir.dt.bfloat16
MAX_OP = mybir.AluOpType.max
MULT_OP = mybir.AluOpType.mult
COPY_FN = mybir.ActivationFunctionType.Copy

LAST_EXEC_TIME_NS = None
LAST_RESULTS = None


def _install_ntff_hook() -> bool:
    """Provide the antenv.axon_hooks shim this image lacks, so
    run_bass_kernel_spmd(trace=True) can capture an NTFF profile."""
    import types

    try:
        from antenv.axon_hooks import get_axon_ntff_profile_hook  # noqa: F401

        return True
    except ImportError:
        pass
    try:
        import antenv
        from trn_agent_boot.trn_boot import _ntff_profile_via_ctypes

        mod = types.ModuleType("antenv.axon_hooks")
        holder = {"h": None}
        mod.set_axon_ntff_profile_hook = lambda h: holder.__setitem__("h", h)
        mod.get_axon_ntff_profile_hook = lambda: holder["h"]
        sys.modules["antenv.axon_hooks"] = mod
        antenv.axon_hooks = mod
        mod.set_axon_ntff_profile_hook(
            _ntff_profile_via_ctypes("/opt/axon/libaxon_pjrt.so")
        )
        return holder["h"] is not None
    except Exception:
        return False


def _build():
    nc = bacc.Bacc(None, target_bir_lowering=False)

    lhs_ext = nc.declare_dram_parameter("lhs", [SHARD, D], F32, isOutput=False)
    rhs_ext = nc.declare_dram_parameter("rhs", [D, F], F32, isOutput=False)
    out_ext = nc.declare_dram_parameter("out", [SHARD, F], F32, isOutput=True)

    with tile.TileContext(nc) as tc:
        with (
            tc.tile_pool(name="singles", bufs=1) as singles,
            tc.tile_pool(name="smalls", bufs=8) as smalls,
            tc.tile_pool(name="xs", bufs=4) as xs_pool,
            tc.tile_pool(name="ts", bufs=3) as ts_pool,
            tc.tile_pool(name="qs", bufs=3) as qs_pool,
            tc.tile_pool(name="qts", bufs=3) as qts_pool,
            tc.tile_pool(name="os", bufs=4) as os_pool,
            tc.tile_pool(name="psum_qt", bufs=2, space="PSUM") as psum_qt,
            tc.tile_pool(name="psum_acc", bufs=2, space="PSUM") as psum_acc,
        ):
            id_f32 = singles.tile([P, P], F32)
            make_identity(nc, id_f32)
            id_bf16 = singles.tile([P, P], BF16)
            make_identity(nc, id_bf16)

            # ---------------- one-time rhs quantization ----------------
            # w_sb[p, k, f] = W[k*128+p, f]
            w_sb = singles.tile([P, KC, F], F32)
            nc.sync.dma_start(
                out=w_sb[:], in_=rhs_ext[:].rearrange("(k p) f -> p k f", p=P)
            )
            # quantized weight, natural [K, F] layout: w_q[k][p, f] = q(W)[k*128+p, f]
            w_q = [
                singles.tile([P, F], BF16, tag=f"w_q{k}", name=f"w_q{k}")
                for k in range(KC)
            ]
            # per-column scales as a broadcast row
            sw_row = singles.tile([1, F], F32)
            # RS[p, f] = col_scale[f] for all p
            rs_bcast = singles.tile([P, F], F32)

            for fc in range(KC):  # F split into 4 chunks of 128
                ps = psum_qt.tile([P, KC, P], F32, tag="qt")
                for k in range(KC):
                    # W[K-chunk k, F-chunk fc] block transposed -> [F-part, K-free]
                    nc.tensor.transpose(
                        ps[:, k, :],
                        w_sb[:, k, fc * P : (fc + 1) * P],
                        id_f32,
                    )
                wT = singles.tile([P, KC, P], F32, tag=f"wT{fc}")
                nc.scalar.copy(wT[:], ps[:])
                amax_w = smalls.tile([P, 1], F32, tag="amax_w")
                nc.vector.tensor_reduce(
                    amax_w,
                    wT[:],
                    axis=mybir.AxisListType.XY,
                    op=MAX_OP,
                    apply_absolute_value=True,
                )
                s_w = smalls.tile([P, 1], F32, tag="s_w")
                nc.vector.tensor_scalar(
                    s_w, amax_w, 1.0 / QMAX, 1e-38, MULT_OP, MAX_OP
                )
                ivs_w = smalls.tile([P, 1], F32, tag="ivs_w")
                nc.vector.reciprocal(ivs_w, s_w)
                t_w = ts_pool.tile([P, KC, P], F32, tag="t1")
                nc.scalar.activation(
                    t_w[:], wT[:], COPY_FN, bias=C_MAGIC, scale=ivs_w
                )
                q_wT = qs_pool.tile([P, KC, P], BF16, tag="q")
                nc.scalar.activation(q_wT[:], t_w[:], COPY_FN, bias=-C_MAGIC)
                for k in range(KC):
                    psb = psum_acc.tile([P, P], BF16, tag="acc")
                    nc.tensor.transpose(psb[:], q_wT[:, k, :], id_bf16)
                    nc.scalar.copy(w_q[k][:, fc * P : (fc + 1) * P], psb[:])
                # col-scale row chunk: [128,1] -> [1,128]
                ps_row = psum_acc.tile([1, P], F32, tag="acc")
                nc.tensor.transpose(ps_row[:], s_w, id_f32)
                nc.scalar.copy(sw_row[0:1, fc * P : (fc + 1) * P], ps_row[:])

            # broadcast col scales across partitions: ones[1,128].T @ sw_row[1,F]
            ones_row = singles.tile([1, P], F32)
            nc.vector.memset(ones_row[:], 1.0)
            ps_bc = psum_acc.tile([P, F], F32, tag="acc")
            nc.tensor.matmul(ps_bc[:], ones_row[:], sw_row[:], start=True, stop=True)
            nc.scalar.copy(rs_bcast[:], ps_bc[:])

            # ---------------- main loop: 64 row-tiles ----------------
            for i in range(N_TILES):
                x = xs_pool.tile([P, D], F32, tag="x")
                nc.sync.dma_start(out=x[:], in_=lhs_ext[i * P : (i + 1) * P, :])

                amax = smalls.tile([P, 1], F32, tag="amax")
                nc.vector.tensor_reduce(
                    amax,
                    x[:],
                    axis=mybir.AxisListType.X,
                    op=MAX_OP,
                    apply_absolute_value=True,
                )
                s = smalls.tile([P, 1], F32, tag="s")
                nc.vector.tensor_scalar(s, amax, 1.0 / QMAX, 1e-38, MULT_OP, MAX_OP)
                ivs = smalls.tile([P, 1], F32, tag="ivs")
                nc.vector.reciprocal(ivs, s)

                t1 = ts_pool.tile([P, D], F32, tag="t1")
                nc.scalar.activation(t1[:], x[:], COPY_FN, bias=C_MAGIC, scale=ivs)
                q = qs_pool.tile([P, D], BF16, tag="q")
                nc.scalar.activation(q[:], t1[:], COPY_FN, bias=-C_MAGIC)

                qt_ps = psum_qt.tile([P, KC, P], BF16, tag="qt")
                for k in range(KC):
                    nc.tensor.transpose(
                        qt_ps[:, k, :], q[:, k * P : (k + 1) * P], id_bf16
                    )
                qt = qts_pool.tile([P, KC, P], BF16, tag="qt_sb")
                nc.scalar.copy(qt[:], qt_ps[:])

                acc = psum_acc.tile([P, F], F32, tag="acc")
                for k in range(KC):
                    nc.tensor.matmul(
                        acc[:],
                        qt[:, k, :],
                        w_q[k][:],
                        start=(k == 0),
                        stop=(k == KC - 1),
                    )

                o = os_pool.tile([P, F], F32, tag="o")
                nc.vector.scalar_tensor_tensor(
                    o[:], acc[:], s, rs_bcast[:], MULT_OP, MULT_OP
                )
                nc.sync.dma_start(out=out_ext[i * P : (i + 1) * P, :], in_=o[:])

    nc.compile()
    return nc


_NC_CACHE = None


def kernel(lhs: np.ndarray, rhs: np.ndarray) -> np.ndarray:
    global LAST_EXEC_TIME_NS, LAST_RESULTS, _NC_CACHE

    lhs = np.ascontiguousarray(np.asarray(lhs, dtype=np.float32))
    rhs = np.ascontiguousarray(np.asarray(rhs, dtype=np.float32))
    flat = lhs.reshape(ROWS, D)

    if _NC_CACHE is None:
        _NC_CACHE = _build()
    nc = _NC_CACHE

    in_maps = [
        {
            "lhs": np.ascontiguousarray(flat[i * SHARD : (i + 1) * SHARD]),
            "rhs": rhs,
        }
        for i in range(N_CORES)
    ]

    trace = bool(os.environ.get("KERNEL_TRACE"))
    if trace:
        trace = _install_ntff_hook()
    res = run_bass_kernel_spmd(
        nc, in_maps, core_ids=list(range(N_CORES)), trace=trace
    )
    LAST_EXEC_TIME_NS = res.exec_time_ns
    LAST_RESULTS = res

    out = np.concatenate([res.results[i]["out"] for i in range(N_CORES)], axis=0)
    return out.reshape(B, S, F).astype(np.float32)
